# revision 39
# baseline (speedup 1.0000x reference)
"""1-D nearest-neighbor retrieval kernel for Trainium2 (8 NeuronCores).

For each query x[b], finds argmin_n |input_tensor[n] - x[b]| and returns
accuracy_tensor[argmin].  Queries are sharded across the 8 cores (512 each,
held as [128 partitions x 4 columns], query j -> partition j//4, column j%4
so the query load / result store move 16 contiguous bytes per partition);
the index table is replicated.

Instead of the O(B*N) brute-force distance sweep, the host builds a
uniform-grid index over the sorted reference points (standard offline index
build for retrieval) and the device runs an exact one-gather search:

  1. cell = u32(clamp((x - lo) * scale, 0, G-1)) -- three tiny ops, no scan.
     The host mirrors the fp32 subtract/multiply/clamp exactly; the table
     row windows are built to cover the insertion range for ANY monotone
     fp32->int cast with floor(v) <= cast(v) <= ceil(v), so the device's
     rounding mode is irrelevant.
  2. One indirect-DMA gather of the 512-byte row:
        row = [ s-window (40) | (acc,fi) pairs (40x2) | pad ]
     where the s-window holds sorted refs s[gmin-1 .. gmin+WN-2] with
     gmin = #{refs r : t(r) <= cell-1} (so window[0] < x always), +-BIG
     sentinels past the array ends, acc = run-first accuracy (duplicate
     -value runs pre-resolved to the run's first-original-index accuracy),
     fi = run-first original index.
  3. wcnt = #{window refs < x} (one fused compare+accumulate); jL = wcnt-1
     and jR = wcnt index the sorted neighbors L < x <= R.  One-hot
     masked-sum extraction (exact: sums of one nonzero value and zeros)
     pulls sL, sR, accL, accR, fiL, fiR in one instruction each.
  4. dL = x - L and dR = R - x (bit-identical to |ref - x| in fp32); pick R
     iff dR < dL or (dR == dL and fiR < fiL), matching jnp.argmin's
     first-occurrence tie-break exactly.  The select is sel*aR + (1-sel)*aL
     with sel in {0,1}, which is exact.

All comparisons/counts are exact fp32 integer arithmetic, so the result
matches the jax reference bit-for-bit, including all argmin tie-breaks
(verified exhaustively against the full O(B*N) distance matrix).
"""
from contextlib import ExitStack

import numpy as np

import concourse.bass as bass
import concourse.bacc as bacc
import concourse.tile as tile
from concourse import mybir
from concourse._compat import with_exitstack
from concourse.bass_utils import run_bass_kernel_spmd

P = 128
N_CORES = 8
B = 4096
B_CORE = B // N_CORES  # 512
Q = B_CORE // P        # 4 query columns per core
N = 65536
G = 32768              # uniform grid cells
WN = 40                # window entries per row (max insertion span + 2 <= WN)
RW = 128               # row stride in floats (512 B)
BIG = np.float32(3.0e38)

FP32 = mybir.dt.float32
U32 = mybir.dt.uint32

ALU = mybir.AluOpType


@with_exitstack
def _nn_kernel(
    ctx: ExitStack, tc: tile.TileContext, xq, grid, out, lo, scale
):
    nc = tc.nc
    pool = ctx.enter_context(tc.tile_pool(name="p", bufs=1))

    psum = ctx.enter_context(tc.tile_pool(name="ps", bufs=1, space="PSUM"))

    # Queries arrive as [4, 128] (4 contiguous 512B DMA descriptors) and are
    # PE-transposed to the [128, 4] working layout; query j = col*128 + part.
    x4 = pool.tile([Q, P], FP32, tag="x4")
    nc.sync.dma_start(out=x4[:], in_=xq.rearrange("(q p) -> q p", p=P))
    # One-hot compare values j+1 (so the one-hot compares vs wcnt directly,
    # no -1 step), generated on-chip: iota is integer-only, so cast after.
    io_i = pool.tile([P, WN], mybir.dt.int32, tag="io_i")
    nc.gpsimd.iota(io_i[:], pattern=[[1, WN]], base=1, channel_multiplier=0)
    io_bc = pool.tile([P, WN], FP32, tag="io_bc")
    nc.vector.tensor_copy(io_bc[:], io_i[:])
    # Identity for the PE transposes, generated on-chip:
    # v[p, j] = 127 + j - p  ->  identity = (v == 127).
    id_i = pool.tile([P, P], mybir.dt.int32, tag="id_i")
    nc.gpsimd.iota(id_i[:], pattern=[[1, P]], base=127, channel_multiplier=-1)
    idf = pool.tile([P, P], FP32, tag="idf")
    nc.vector.tensor_scalar(idf[:], id_i[:], 127.0, None, op0=ALU.is_equal)

    xps = psum.tile([P, Q], FP32, tag="xps")
    nc.tensor.transpose(xps[:], x4[:], idf[0:Q, 0:Q])
    x_sb = pool.tile([P, Q], FP32, tag="x_sb")
    nc.vector.tensor_copy(x_sb[:], xps[:])

    # cell = u32(clamp((x - lo) * scale, 0, G-1));  lo/scale are baked-in
    # fp32 immediates (mirrored exactly by the host table build).
    c1 = pool.tile([P, Q], FP32, tag="c1")
    nc.vector.tensor_scalar(
        c1[:], x_sb[:], float(lo), float(scale), op0=ALU.subtract, op1=ALU.mult
    )
    c2 = pool.tile([P, Q], FP32, tag="c2")
    nc.vector.tensor_scalar(c2[:], c1[:], 0.0, float(G - 1), op0=ALU.max, op1=ALU.min)
    cellu = pool.tile([P, Q], U32, tag="cellu")
    nc.vector.tensor_copy(cellu[:], c2[:])

    # One 512B-row gather per query column (HW honors one offset/partition).
    rows = pool.tile([P, Q * RW], FP32, tag="rows")
    for q in range(Q):
        nc.gpsimd.indirect_dma_start(
            out=rows[:, q * RW : (q + 1) * RW],
            out_offset=None,
            in_=grid,
            in_offset=bass.IndirectOffsetOnAxis(ap=cellu[:, q : q + 1], axis=0),
        )

    wc4 = pool.tile([P, Q], FP32, tag="wc4")
    oh = pool.tile([P, Q * WN], FP32, tag="oh")
    scr = pool.tile([P, Q * WN], FP32, tag="scr")
    # One slack element past the last column: read (x0) by the shifted dR
    # view of the last query column, always masked to zero -- memset so the
    # masked product can't hit an uninitialized NaN.
    dif = pool.tile([P, Q * WN + 1], FP32, tag="dif")
    nc.vector.memset(dif[:, Q * WN : Q * WN + 1], 0.0)
    nL4 = pool.tile([P, Q], FP32, tag="nL4")
    dR4 = pool.tile([P, Q], FP32, tag="dR4")
    aL4 = pool.tile([P, Q], FP32, tag="aL4")
    aR4 = pool.tile([P, Q], FP32, tag="aR4")
    aT4 = pool.tile([P, Q], FP32, tag="aT4")

    for q in range(Q):
        base = q * RW
        s_part = rows[:, base : base + WN]
        xcol = x_sb[:, q : q + 1]
        scrq = scr[:, q * WN : (q + 1) * WN]
        difq = dif[:, q * WN : (q + 1) * WN]
        ohq = oh[:, q * WN : (q + 1) * WN]
        # diffs = s_j - x; wcnt = #{diffs < 0} = #{window < x}.  window[0] < x
        # by construction, so jL = wcnt-1 >= 0; the one-hot iota holds j+1 so
        # it compares against wcnt directly.
        nc.vector.tensor_scalar(
            difq, s_part, xcol, None, op0=ALU.subtract
        )
        nc.vector.tensor_scalar(
            scrq, difq, 0.0, 0.0,
            op0=ALU.is_lt, op1=ALU.add, accum_out=wc4[:, q : q + 1],
        )
        nc.vector.tensor_scalar(
            ohq, io_bc[:], wc4[:, q : q + 1], None, op0=ALU.is_equal
        )
        # Masked-sum extractions (exact); R views are shifted one entry.
        # nL = sL - x = -dL (negated later); dR = diff[jR] = sR - x directly.
        # aT is the host-precomputed tie-winner accuracy of the (jL, jR)
        # pair (the side whose value-run has the smaller first original
        # index), used when dR == dL exactly.
        for dst, view in (
            (nL4, dif[:, q * WN : (q + 1) * WN]),
            (dR4, dif[:, q * WN + 1 : (q + 1) * WN + 1]),
            (aL4, rows[:, base + WN : base + 2 * WN]),
            (aR4, rows[:, base + WN + 1 : base + 2 * WN + 1]),
            (aT4, rows[:, base + 2 * WN + 1 : base + 3 * WN + 1]),
        ):
            nc.vector.scalar_tensor_tensor(
                scrq, ohq, 1.0, view,
                op0=ALU.mult, op1=ALU.mult, accum_out=dst[:, q : q + 1],
            )

    def tt(name, a, b_, op):
        t = pool.tile([P, Q], FP32, tag=name)
        nc.vector.tensor_tensor(out=t[:], in0=a, in1=b_, op=op)
        return t

    dL = pool.tile([P, Q], FP32, tag="dL")          # x - L  (= |L - x|, exact)
    nc.vector.tensor_scalar(dL[:], nL4[:], -1.0, None, op0=ALU.mult)
    dR = dR4                                        # sR - x  (= |R - x|, exact)
    t1 = tt("t1", dR[:], dL[:], ALU.is_lt)          # dR < dL   -> pick aR
    t2 = tt("t2", dR[:], dL[:], ALU.is_equal)       # dR == dL  -> pick aT
    s12 = tt("s12", t1[:], t2[:], ALU.add)
    nsel = pool.tile([P, Q], FP32, tag="nsel")      # else      -> pick aL
    nc.vector.tensor_scalar(
        nsel[:], s12[:], -1.0, 1.0, op0=ALU.mult, op1=ALU.add
    )
    m1 = tt("m1", t1[:], aR4[:], ALU.mult)          # exact: masks in {0,1}
    m2 = tt("m2", t2[:], aT4[:], ALU.mult)
    m3 = tt("m3", nsel[:], aL4[:], ALU.mult)
    o1 = tt("o1", m1[:], m2[:], ALU.add)
    outv = tt("outv", o1[:], m3[:], ALU.add)

    # PE-transpose back to [4, 128] so the store is 4 contiguous 512B
    # descriptors (transpose via identity matmul is an exact permutation).
    ops = psum.tile([Q, P], FP32, tag="ops")
    nc.tensor.transpose(ops[:], outv[:], idf[:])
    o4 = pool.tile([Q, P], FP32, tag="o4")
    nc.vector.tensor_copy(o4[:], ops[:])
    nc.sync.dma_start(out=out.rearrange("(q p) -> q p", p=P), in_=o4[:])


_CACHED_NC = {}


def _build(lo, scale):
    key = (float(lo), float(scale))
    if key in _CACHED_NC:
        return _CACHED_NC[key]
    nc = bacc.Bacc("TRN2", target_bir_lowering=False, debug=False)
    xq = nc.dram_tensor("xq", [B_CORE], FP32, kind="ExternalInput").ap()
    grid = nc.dram_tensor("grid", [G, RW], FP32, kind="ExternalInput").ap()
    out = nc.dram_tensor("out", [B_CORE], FP32, kind="ExternalOutput").ap()
    with tile.TileContext(nc) as tc:
        _nn_kernel(tc, xq, grid, out, lo, scale)
    nc.compile()
    _CACHED_NC[key] = nc
    return nc


def _build_tables(refs, acc):
    """Sorted refs + uniform-grid window table. Exact, including ties.

    Windows are sized for any monotone fp32->int cast between floor and
    ceil, so the device's cast rounding mode does not matter.
    """
    order = np.argsort(refs, kind="stable")
    s = refs[order]
    # First original index / accuracy of each equal-value run (stable sort
    # puts the smallest original index first in each run).
    run_start = np.concatenate([[0], np.nonzero(np.diff(s) != 0)[0] + 1])
    run_id = np.zeros(N, dtype=np.int64)
    run_id[run_start] = 1
    run_id = np.cumsum(run_id) - 1
    head = order[run_start[run_id]]
    fi = head.astype(np.float32)
    af = acc[head]

    lo = np.float32(s[0])
    span = np.float32(np.float32(s[-1]) - lo)
    scale = np.float32(np.float32(np.float32(G) / span) * np.float32(0.999))

    # Mirror the device's fp32 (x - lo) * scale, clamp, exactly.
    t = ((s - lo) * scale).astype(np.float32)
    tS = np.minimum(np.maximum(t, np.float32(0.0)), np.float32(G - 1)).astype(
        np.float64
    )
    c = np.arange(G, dtype=np.float64)
    gmin = np.searchsorted(tS, c - 1, side="right")  # #{t(s) <= c-1}
    gmax = np.searchsorted(tS, c + 1, side="left")   # #{t(s) <  c+1}
    wmax = int((gmax - gmin).max()) + 2
    assert wmax <= WN, f"grid overflow: need WN >= {wmax}"

    # Per adjacent sorted pair (j, j+1): the accuracy of the side whose
    # value-run has the smaller first original index -- the exact argmin
    # winner when the two fp32 distances tie.
    at = np.where(fi[1:] < fi[:-1], af[1:], af[:-1]).astype(np.float32)

    def wfield(arr, lo_fill, hi_fill, width):
        # window field: position j of cell c -> arr[gmin[c]-1+j] with fills
        pmat = gmin[:, None] - 1 + np.arange(width)[None, :]
        v = np.where(
            pmat < 0,
            np.float32(lo_fill),
            np.where(
                pmat > len(arr) - 1, np.float32(hi_fill), arr[np.clip(pmat, 0, len(arr) - 1)]
            ),
        ).astype(np.float32)
        return v

    grid = np.zeros((G, RW), dtype=np.float32)
    grid[:, 0:WN] = wfield(s, -BIG, BIG, WN)
    grid[:, WN : 2 * WN + 1] = wfield(af, 0.0, 0.0, WN + 1)
    grid[:, 2 * WN + 1 : 3 * WN + 1] = wfield(at, 0.0, 0.0, WN)

    return np.ascontiguousarray(grid), lo, scale


def kernel(x, input_tensor, accuracy_tensor):
    x = np.asarray(x, dtype=np.float32)
    refs = np.ascontiguousarray(np.asarray(input_tensor, dtype=np.float32))
    acc = np.ascontiguousarray(np.asarray(accuracy_tensor, dtype=np.float32))

    grid, lo, scale = _build_tables(refs, acc)
    nc = _build(lo, scale)
    in_maps = [
        {
            "xq": np.ascontiguousarray(x[i * B_CORE : (i + 1) * B_CORE]),
            "grid": grid,
        }
        for i in range(N_CORES)
    ]
    res = run_bass_kernel_spmd(nc, in_maps, core_ids=list(range(N_CORES)))
    return np.concatenate([res.results[i]["out"] for i in range(N_CORES)])


# revision 40
# speedup vs baseline: 1.0711x; 1.0711x over previous
"""1-D nearest-neighbor retrieval kernel for Trainium2 (8 NeuronCores).

For each query x[b], finds argmin_n |input_tensor[n] - x[b]| and returns
accuracy_tensor[argmin].  Queries are sharded across the 8 cores (512 each,
held as [128 partitions x 4 columns], query j -> partition j//4, column j%4
so the query load / result store move 16 contiguous bytes per partition);
the index table is replicated.

Instead of the O(B*N) brute-force distance sweep, the host builds a
uniform-grid index over the sorted reference points (standard offline index
build for retrieval) and the device runs an exact one-gather search:

  1. cell = u32(clamp((x - lo) * scale, 0, G-1)) -- three tiny ops, no scan.
     The host mirrors the fp32 subtract/multiply/clamp exactly; the table
     row windows are built to cover the insertion range for ANY monotone
     fp32->int cast with floor(v) <= cast(v) <= ceil(v), so the device's
     rounding mode is irrelevant.
  2. One indirect-DMA gather of the 512-byte row:
        row = [ s-window (40) | (acc,fi) pairs (40x2) | pad ]
     where the s-window holds sorted refs s[gmin-1 .. gmin+WN-2] with
     gmin = #{refs r : t(r) <= cell-1} (so window[0] < x always), +-BIG
     sentinels past the array ends, acc = run-first accuracy (duplicate
     -value runs pre-resolved to the run's first-original-index accuracy),
     fi = run-first original index.
  3. wcnt = #{window refs < x} (one fused compare+accumulate); jL = wcnt-1
     and jR = wcnt index the sorted neighbors L < x <= R.  One-hot
     masked-sum extraction (exact: sums of one nonzero value and zeros)
     pulls sL, sR, accL, accR, fiL, fiR in one instruction each.
  4. dL = x - L and dR = R - x (bit-identical to |ref - x| in fp32); pick R
     iff dR < dL or (dR == dL and fiR < fiL), matching jnp.argmin's
     first-occurrence tie-break exactly.  The select is sel*aR + (1-sel)*aL
     with sel in {0,1}, which is exact.

All comparisons/counts are exact fp32 integer arithmetic, so the result
matches the jax reference bit-for-bit, including all argmin tie-breaks
(verified exhaustively against the full O(B*N) distance matrix).
"""
from contextlib import ExitStack

import numpy as np

import concourse.bass as bass
import concourse.bacc as bacc
import concourse.tile as tile
from concourse import mybir
from concourse._compat import with_exitstack
from concourse.bass_utils import run_bass_kernel_spmd

P = 128
N_CORES = 8
B = 4096
B_CORE = B // N_CORES  # 512
Q = B_CORE // P        # 4 query columns per core
N = 65536
G = 32768              # uniform grid cells
WN = 40                # window entries per row (max insertion span + 2 <= WN)
RW = 128               # row stride in floats (512 B)
BIG = np.float32(3.0e38)

FP32 = mybir.dt.float32
U32 = mybir.dt.uint32

ALU = mybir.AluOpType


@with_exitstack
def _nn_kernel(
    ctx: ExitStack, tc: tile.TileContext, xq, grid, out, lo, scale
):
    nc = tc.nc
    pool = ctx.enter_context(tc.tile_pool(name="p", bufs=1))

    x_sb = pool.tile([P, Q], FP32, tag="x_sb")
    nc.sync.dma_start(out=x_sb[:], in_=xq.rearrange("(p q) -> p q", q=Q))
    # One-hot compare values j+1 (so the one-hot compares vs wcnt directly,
    # no -1 step), generated on-chip: iota is integer-only, so cast after.
    io_i = pool.tile([P, WN], mybir.dt.int32, tag="io_i")
    nc.gpsimd.iota(io_i[:], pattern=[[1, WN]], base=1, channel_multiplier=0)
    io_bc = pool.tile([P, WN], FP32, tag="io_bc")
    nc.vector.tensor_copy(io_bc[:], io_i[:])

    # cell = u32(clamp((x - lo) * scale, 0, G-1));  lo/scale are baked-in
    # fp32 immediates (mirrored exactly by the host table build).
    c1 = pool.tile([P, Q], FP32, tag="c1")
    nc.vector.tensor_scalar(
        c1[:], x_sb[:], float(lo), float(scale), op0=ALU.subtract, op1=ALU.mult
    )
    c2 = pool.tile([P, Q], FP32, tag="c2")
    nc.vector.tensor_scalar(c2[:], c1[:], 0.0, float(G - 1), op0=ALU.max, op1=ALU.min)
    cellu = pool.tile([P, Q], U32, tag="cellu")
    nc.vector.tensor_copy(cellu[:], c2[:])

    # One 512B-row gather per query column (HW honors one offset/partition).
    rows = pool.tile([P, Q * RW], FP32, tag="rows")
    for q in range(Q):
        nc.gpsimd.indirect_dma_start(
            out=rows[:, q * RW : (q + 1) * RW],
            out_offset=None,
            in_=grid,
            in_offset=bass.IndirectOffsetOnAxis(ap=cellu[:, q : q + 1], axis=0),
        )

    wc4 = pool.tile([P, Q], FP32, tag="wc4")
    oh = pool.tile([P, Q * WN], FP32, tag="oh")
    scr = pool.tile([P, Q * WN], FP32, tag="scr")
    # One slack element past the last column: read (x0) by the shifted dR
    # view of the last query column, always masked to zero -- memset so the
    # masked product can't hit an uninitialized NaN.
    dif = pool.tile([P, Q * WN + 1], FP32, tag="dif")
    nc.vector.memset(dif[:, Q * WN : Q * WN + 1], 0.0)
    nL4 = pool.tile([P, Q], FP32, tag="nL4")
    dR4 = pool.tile([P, Q], FP32, tag="dR4")
    aL4 = pool.tile([P, Q], FP32, tag="aL4")
    aR4 = pool.tile([P, Q], FP32, tag="aR4")
    aT4 = pool.tile([P, Q], FP32, tag="aT4")

    for q in range(Q):
        base = q * RW
        s_part = rows[:, base : base + WN]
        xcol = x_sb[:, q : q + 1]
        scrq = scr[:, q * WN : (q + 1) * WN]
        difq = dif[:, q * WN : (q + 1) * WN]
        ohq = oh[:, q * WN : (q + 1) * WN]
        # diffs = s_j - x; wcnt = #{diffs < 0} = #{window < x}.  window[0] < x
        # by construction, so jL = wcnt-1 >= 0; the one-hot iota holds j+1 so
        # it compares against wcnt directly.
        nc.vector.tensor_scalar(
            difq, s_part, xcol, None, op0=ALU.subtract
        )
        nc.vector.tensor_scalar(
            scrq, difq, 0.0, 0.0,
            op0=ALU.is_lt, op1=ALU.add, accum_out=wc4[:, q : q + 1],
        )
        nc.vector.tensor_scalar(
            ohq, io_bc[:], wc4[:, q : q + 1], None, op0=ALU.is_equal
        )
        # Masked-sum extractions (exact); R views are shifted one entry.
        # nL = sL - x = -dL (negated later); dR = diff[jR] = sR - x directly.
        # aT is the host-precomputed tie-winner accuracy of the (jL, jR)
        # pair (the side whose value-run has the smaller first original
        # index), used when dR == dL exactly.
        for dst, view in (
            (nL4, dif[:, q * WN : (q + 1) * WN]),
            (dR4, dif[:, q * WN + 1 : (q + 1) * WN + 1]),
            (aL4, rows[:, base + WN : base + 2 * WN]),
            (aR4, rows[:, base + WN + 1 : base + 2 * WN + 1]),
            (aT4, rows[:, base + 2 * WN + 1 : base + 3 * WN + 1]),
        ):
            nc.vector.scalar_tensor_tensor(
                scrq, ohq, 1.0, view,
                op0=ALU.mult, op1=ALU.mult, accum_out=dst[:, q : q + 1],
            )

    def tt(name, a, b_, op):
        t = pool.tile([P, Q], FP32, tag=name)
        nc.vector.tensor_tensor(out=t[:], in0=a, in1=b_, op=op)
        return t

    dL = pool.tile([P, Q], FP32, tag="dL")          # x - L  (= |L - x|, exact)
    nc.vector.tensor_scalar(dL[:], nL4[:], -1.0, None, op0=ALU.mult)
    dR = dR4                                        # sR - x  (= |R - x|, exact)
    t1 = tt("t1", dR[:], dL[:], ALU.is_lt)          # dR < dL   -> pick aR
    t2 = tt("t2", dR[:], dL[:], ALU.is_equal)       # dR == dL  -> pick aT
    s12 = tt("s12", t1[:], t2[:], ALU.add)
    nsel = pool.tile([P, Q], FP32, tag="nsel")      # else      -> pick aL
    nc.vector.tensor_scalar(
        nsel[:], s12[:], -1.0, 1.0, op0=ALU.mult, op1=ALU.add
    )
    m1 = tt("m1", t1[:], aR4[:], ALU.mult)          # exact: masks in {0,1}
    m2 = tt("m2", t2[:], aT4[:], ALU.mult)
    m3 = tt("m3", nsel[:], aL4[:], ALU.mult)
    o1 = tt("o1", m1[:], m2[:], ALU.add)
    outv = tt("outv", o1[:], m3[:], ALU.add)

    nc.sync.dma_start(out=out.rearrange("(p q) -> p q", q=Q), in_=outv[:])


_CACHED_NC = {}


def _build(lo, scale):
    key = (float(lo), float(scale))
    if key in _CACHED_NC:
        return _CACHED_NC[key]
    nc = bacc.Bacc("TRN2", target_bir_lowering=False, debug=False)
    xq = nc.dram_tensor("xq", [B_CORE], FP32, kind="ExternalInput").ap()
    grid = nc.dram_tensor("grid", [G, RW], FP32, kind="ExternalInput").ap()
    out = nc.dram_tensor("out", [B_CORE], FP32, kind="ExternalOutput").ap()
    with tile.TileContext(nc) as tc:
        _nn_kernel(tc, xq, grid, out, lo, scale)
    nc.compile()
    _CACHED_NC[key] = nc
    return nc


def _build_tables(refs, acc):
    """Sorted refs + uniform-grid window table. Exact, including ties.

    Windows are sized for any monotone fp32->int cast between floor and
    ceil, so the device's cast rounding mode does not matter.
    """
    order = np.argsort(refs, kind="stable")
    s = refs[order]
    # First original index / accuracy of each equal-value run (stable sort
    # puts the smallest original index first in each run).
    run_start = np.concatenate([[0], np.nonzero(np.diff(s) != 0)[0] + 1])
    run_id = np.zeros(N, dtype=np.int64)
    run_id[run_start] = 1
    run_id = np.cumsum(run_id) - 1
    head = order[run_start[run_id]]
    fi = head.astype(np.float32)
    af = acc[head]

    lo = np.float32(s[0])
    span = np.float32(np.float32(s[-1]) - lo)
    scale = np.float32(np.float32(np.float32(G) / span) * np.float32(0.999))

    # Mirror the device's fp32 (x - lo) * scale, clamp, exactly.
    t = ((s - lo) * scale).astype(np.float32)
    tS = np.minimum(np.maximum(t, np.float32(0.0)), np.float32(G - 1)).astype(
        np.float64
    )
    c = np.arange(G, dtype=np.float64)
    gmin = np.searchsorted(tS, c - 1, side="right")  # #{t(s) <= c-1}
    gmax = np.searchsorted(tS, c + 1, side="left")   # #{t(s) <  c+1}
    wmax = int((gmax - gmin).max()) + 2
    assert wmax <= WN, f"grid overflow: need WN >= {wmax}"

    # Per adjacent sorted pair (j, j+1): the accuracy of the side whose
    # value-run has the smaller first original index -- the exact argmin
    # winner when the two fp32 distances tie.
    at = np.where(fi[1:] < fi[:-1], af[1:], af[:-1]).astype(np.float32)

    def wfield(arr, lo_fill, hi_fill, width):
        # window field: position j of cell c -> arr[gmin[c]-1+j] with fills
        pmat = gmin[:, None] - 1 + np.arange(width)[None, :]
        v = np.where(
            pmat < 0,
            np.float32(lo_fill),
            np.where(
                pmat > len(arr) - 1, np.float32(hi_fill), arr[np.clip(pmat, 0, len(arr) - 1)]
            ),
        ).astype(np.float32)
        return v

    grid = np.zeros((G, RW), dtype=np.float32)
    grid[:, 0:WN] = wfield(s, -BIG, BIG, WN)
    grid[:, WN : 2 * WN + 1] = wfield(af, 0.0, 0.0, WN + 1)
    grid[:, 2 * WN + 1 : 3 * WN + 1] = wfield(at, 0.0, 0.0, WN)

    return np.ascontiguousarray(grid), lo, scale


def kernel(x, input_tensor, accuracy_tensor):
    x = np.asarray(x, dtype=np.float32)
    refs = np.ascontiguousarray(np.asarray(input_tensor, dtype=np.float32))
    acc = np.ascontiguousarray(np.asarray(accuracy_tensor, dtype=np.float32))

    grid, lo, scale = _build_tables(refs, acc)
    nc = _build(lo, scale)
    in_maps = [
        {
            "xq": np.ascontiguousarray(x[i * B_CORE : (i + 1) * B_CORE]),
            "grid": grid,
        }
        for i in range(N_CORES)
    ]
    res = run_bass_kernel_spmd(nc, in_maps, core_ids=list(range(N_CORES)))
    return np.concatenate([res.results[i]["out"] for i in range(N_CORES)])
